# revision 12
# baseline (speedup 1.0000x reference)
"""Trainium2 Bass kernel for an LIF spiking-neuron bank (FMFMNeuronBank).

Reference semantics (see problem statement):
    cur[t,b,n] = spike_seq[t,b,0]*W[n,0] + spike_seq[t,b,1]*W[n,1]
    mem_t = 0.9*mem_{t-1} + cur_t - spk_{t-1}          (f32, this exact assoc.)
    spk_t = (mem_t > 1.0)
    out[t,b,n] = spk_t                                  [2048, 512, 128] f32

Distribution: data-parallel over batch B across 8 cores (64 batch rows each).
Per-core layout: partitions = neuron dim N (128), free dim = local batch (64).

Per-core engine pipeline:
  PE    : cur = W6.T @ S6 as a K=6 bf16 matmul into PSUM. Weights are split
          into three bf16 terms each (hi/mid/lo) so the f32 weight values are
          reconstructed exactly; spikes are 0/1 so every product is exact.
  ACT   : bulk-copies cur chunks PSUM -> SBUF.
  DVE   : one fused custom op per timestep (the serial chain):
              m_t = (0.9*m_{t-1} + cur_t) - (m_{t-1} > 1)
          This works because the spike subtracted at step t is an elementwise
          function of the *previous* membrane. Membrane trajectory goes to a
          ring buffer in SBUF.
  GPSIMD: bulk-thresholds trajectory chunks into 0/1 spike tiles.
  DMA   : streams spike tiles to DRAM in dense 2 MB transfers ([N, T, B']
          layout so every partition writes contiguous runs).

The f32 rounding of this pipeline was validated against the jax-CPU reference
(zero mismatching spikes over all 134M outputs).
"""

import numpy as np
import ml_dtypes

import concourse.bass as bass
import concourse.mybir as mybir
import concourse.tile as tile
from concourse import bacc
from concourse.bass_utils import run_bass_kernel_spmd

# ------------------------------------------------------------------ problem
T, B, N = 2048, 512, 128
NCORES = 8
BP = B // NCORES          # local batch per core = 64
BETA = 0.9
THR = 1.0

# ------------------------------------------------------------------ tiling
R = 256                   # membrane-trajectory ring slots (t)
G = 64                    # timesteps per bulk-spike/DMA group
CH = 8                    # timesteps per PSUM matmul chunk (8*64 = 512 free)
RH = 128                  # timesteps per rhs DRAM->SBUF load
F = CH * BP               # matmul free size = 512

_FP32 = mybir.dt.float32
_BF16 = mybir.dt.bfloat16


# --------------------------------------------------- custom DVE op: LIF step
def _register_lif_op():
    """Register the fused LIF-step op:  out = (in0*C0 + in1) - (in0 > 1)."""
    import concourse.dve_ops as dve_ops
    from concourse.dve_spec import Spec, Src0, Src1, C0, One, lower, _has_src1
    from concourse.dve_uop import DveOpSpec

    name = "LIF_STEP_ANT"
    if name in dve_ops._SUB_OPCODE_FOR_NAME:
        return next(op for op in dve_ops.OPS if op.name == name)

    spec = Spec(
        body=(Src0 * C0 + Src1) - (Src0 > One),
        reference=lambda in0, in1, s0, s1, imm2: (
            (in0 * np.float32(s0) + in1)
            - (in0 > np.float32(1.0)).astype(np.float32)
        ),
    )
    row = dve_ops._CUSTOM_DVE_ROW_BASE + len(dve_ops.OPS)
    shas = {}
    for ver in ("v3", "v4"):
        d = DveOpSpec(
            name=name, opcode=row, uops=lower(spec, ver=ver),
            rd1_en=_has_src1(spec),
        )
        shas[ver] = d.sha(ver)
    op = dve_ops.DveOp(name, spec, subdim=False, uops_sha=shas)
    dve_ops.OPS.append(op)
    dve_ops._SUB_OPCODE_FOR_NAME[name] = row
    dve_ops.CUSTOM_DVE_SPECS[name] = spec
    return op


def _register_lif_direct_op():
    """Fused LIF step with in-op current computation (constant-w1 case):

        out = (in0*imm2 + (in1*C0 + C1)) - (in0 > 1)

    in0 = mem, in1 = w2 broadcast tile (constant), C0 = s1 column,
    C1 = w1*s0 column (host-premultiplied, exact), imm2 = beta.
    """
    import concourse.dve_ops as dve_ops
    from concourse.dve_spec import (
        Spec, Src0, Src1, C0, C1, C2, One, lower, _has_src1,
    )
    from concourse.dve_uop import DveOpSpec

    name = "LIF_DIRECT_ANT"
    if name in dve_ops._SUB_OPCODE_FOR_NAME:
        return next(op for op in dve_ops.OPS if op.name == name)

    spec = Spec(
        body=(Src0 * C2 + (Src1 * C0 + C1)) - (Src0 > One),
        reference=lambda in0, in1, s0, s1, imm2: (
            (in0 * np.float32(imm2) + (in1 * s0 + s1))
            - (in0 > np.float32(1.0)).astype(np.float32)
        ),
    )
    row = dve_ops._CUSTOM_DVE_ROW_BASE + len(dve_ops.OPS)
    shas = {}
    for ver in ("v3", "v4"):
        d = DveOpSpec(
            name=name, opcode=row, uops=lower(spec, ver=ver),
            rd1_en=_has_src1(spec),
        )
        shas[ver] = d.sha(ver)
    op = dve_ops.DveOp(name, spec, subdim=False, uops_sha=shas)
    dve_ops.OPS.append(op)
    dve_ops._SUB_OPCODE_FOR_NAME[name] = row
    dve_ops.CUSTOM_DVE_SPECS[name] = spec
    return op


# --------------------------------------------------------------- bass build
def _build_program(T=T, variant="normal"):
    flags = set(variant.split("+"))
    lif_op = _register_lif_op()

    nc = bacc.Bacc(
        "TRN2",
        target_bir_lowering=False,
        debug=False,
        enable_asserts=False,
        num_devices=NCORES,
    )

    rhs_dram = nc.dram_tensor("rhs6", [6, T * BP], _BF16, kind="ExternalInput").ap()
    w6_dram = nc.dram_tensor("w6", [6, N], _BF16, kind="ExternalInput").ap()
    out_T = 1 if "tinybuf" in flags else T
    out_dram = nc.dram_tensor("out", [N, out_T, BP], _FP32, kind="ExternalOutput").ap()

    with tile.TileContext(nc) as tc:
        with (
            tc.tile_pool(name="const", bufs=1) as const_pool,
            tc.tile_pool(name="rhs", bufs=2) as rhs_pool,
            tc.tile_pool(name="psum", bufs=4, space="PSUM") as psum_pool,
            tc.tile_pool(name="cur", bufs=8) as cur_pool,
            tc.tile_pool(name="traj", bufs=1) as traj_pool,
            tc.tile_pool(name="spk", bufs=2) as spk_pool,
        ):
            w6_sb = const_pool.tile([6, N], _BF16, tag="w6")
            nc.sync.dma_start(out=w6_sb[:, :], in_=w6_dram[:, :])

            traj = traj_pool.tile([N, R * BP], _FP32, tag="traj")
            # slot R-1 is mem_{-1} = 0
            nc.vector.memset(traj[:, (R - 1) * BP : R * BP], 0.0)

            for rc in range(T // RH):                       # 16 rhs chunks
                rhs_t = rhs_pool.tile([6, RH * BP], _BF16, tag="rhs")
                off = rc * RH * BP
                nc.sync.dma_start(
                    out=rhs_t[:, :], in_=rhs_dram[:, off : off + RH * BP]
                )
                for mc in range(RH // CH):                  # 16 matmuls
                    ps = psum_pool.tile([N, F], _FP32, tag="ps")
                    nc.tensor.matmul(
                        ps[:, :],
                        w6_sb[:, :],
                        rhs_t[:, mc * F : (mc + 1) * F],
                        start=True,
                        stop=True,
                    )
                    cur = cur_pool.tile([N, F], _FP32, tag="cur")
                    nc.scalar.activation(
                        cur[:, :], ps[:, :], mybir.ActivationFunctionType.Copy
                    )
                    for j in range(CH):                     # 8 serial LIF steps
                        t = rc * RH + mc * CH + j
                        slot = t % R
                        prev = (t - 1) % R if "nochain" not in flags else R - 1
                        if "nodve" not in flags:
                            nc.vector._custom_dve(
                                lif_op,
                                out=traj[:, slot * BP : (slot + 1) * BP],
                                in0=traj[:, prev * BP : (prev + 1) * BP],
                                in1=cur[:, j * BP : (j + 1) * BP],
                                s0=BETA,
                            )
                        if (t + 1) % G == 0:
                            g = t // G
                            base = (g * G) % R
                            spk = spk_pool.tile([N, G * BP], _FP32, tag="spk")
                            if "nospike" not in flags:
                                spike_eng = (
                                    nc.gpsimd
                                    if "spike_gpsimd" in flags
                                    else nc.vector
                                )
                                spike_eng.tensor_scalar(
                                    spk[:, :],
                                    traj[:, base * BP : (base + G) * BP],
                                    THR,
                                    None,
                                    mybir.AluOpType.is_gt,
                                )
                            if not flags & {"nodma", "tinybuf", "nospike"}:
                                nc.sync.dma_start(
                                    out=out_dram[:, g * G : (g + 1) * G, :],
                                    in_=spk[:, :].rearrange("p (t b) -> p t b", b=BP),
                                )

    nc.compile()
    return nc


def _build_program_direct(T=T, variant="normal"):
    """Constant-w1 fast path: no PE/ACT/PSUM — the fused DVE op computes the
    input current in-op. Layout: partitions = (n_half, local_b), free = n%64.
    """
    flags = set(variant.split("+"))
    op = _register_lif_direct_op()

    nc = bacc.Bacc(
        "TRN2",
        target_bir_lowering=False,
        debug=False,
        enable_asserts=False,
        num_devices=NCORES,
    )

    # scols: columns [0..T) = s1[t] per partition; [T..2T) = w1*s0[t]
    scols_dram = nc.dram_tensor(
        "scols", [128, 2 * T], _FP32, kind="ExternalInput"
    ).ap()
    w2b_dram = nc.dram_tensor("w2b", [128, BP], _FP32, kind="ExternalInput").ap()
    out_T = 1 if "tinybuf" in flags else T
    out_dram = nc.dram_tensor(
        "out", [128, out_T, BP], _FP32, kind="ExternalOutput"
    ).ap()

    with tile.TileContext(nc) as tc:
        with (
            tc.tile_pool(name="const", bufs=1) as const_pool,
            tc.tile_pool(name="traj", bufs=1) as traj_pool,
            tc.tile_pool(name="spk", bufs=2) as spk_pool,
        ):
            w2b = const_pool.tile([128, BP], _FP32, tag="w2b")
            nc.sync.dma_start(out=w2b[:, :], in_=w2b_dram[:, :])
            scols = const_pool.tile([128, 2 * T], _FP32, tag="scols")
            nc.sync.dma_start(out=scols[:, :], in_=scols_dram[:, :])

            traj = traj_pool.tile([128, R * BP], _FP32, tag="traj")
            nc.vector.memset(traj[:, (R - 1) * BP : R * BP], 0.0)

            for t in range(T):
                slot = t % R
                prev = (t - 1) % R if "nochain" not in flags else R - 1
                if "nodve" not in flags:
                    nc.vector._custom_dve(
                        op,
                        out=traj[:, slot * BP : (slot + 1) * BP],
                        in0=traj[:, prev * BP : (prev + 1) * BP],
                        in1=w2b[:, :],
                        s0=scols[:, t : t + 1],
                        s1=scols[:, T + t : T + t + 1],
                        imm2=BETA,
                    )
                if (t + 1) % G == 0:
                    g = t // G
                    base = (g * G) % R
                    spk = spk_pool.tile([128, G * BP], _FP32, tag="spk")
                    if "nospike" not in flags:
                        nc.vector.tensor_scalar(
                            spk[:, :],
                            traj[:, base * BP : (base + G) * BP],
                            THR,
                            None,
                            mybir.AluOpType.is_gt,
                        )
                    if not flags & {"nodma", "tinybuf", "nospike"}:
                        nc.sync.dma_start(
                            out=out_dram[:, g * G : (g + 1) * G, :],
                            in_=spk[:, :].rearrange("p (t b) -> p t b", b=BP),
                        )

    nc.compile()
    return nc


_PROGRAMS = {}


def _get_program(kind="pe"):
    if kind not in _PROGRAMS:
        _PROGRAMS[kind] = (
            _build_program_direct() if kind == "direct" else _build_program()
        )
    return _PROGRAMS[kind]


# -------------------------------------------------------------- host driver
def _split3_bf16(w: np.ndarray):
    """Exact 3-term bf16 split of f32 values: w == hi + mid + lo (in f32)."""
    w = w.astype(np.float32)
    hi = w.astype(ml_dtypes.bfloat16)
    r1 = (w - hi.astype(np.float32)).astype(np.float32)
    mid = r1.astype(ml_dtypes.bfloat16)
    r2 = (r1 - mid.astype(np.float32)).astype(np.float32)
    lo = r2.astype(ml_dtypes.bfloat16)
    assert np.all(
        hi.astype(np.float32) + mid.astype(np.float32) + lo.astype(np.float32) == w
    ), "bf16 3-term split not exact"
    return hi, mid, lo


def kernel(spike_seq: np.ndarray, W: np.ndarray) -> np.ndarray:
    spike_seq = np.asarray(spike_seq, dtype=np.float32)
    W = np.asarray(W, dtype=np.float32)
    assert spike_seq.shape == (T, B, 2) and W.shape == (N, 2)

    if np.all(W[:, 0] == W[0, 0]):
        return _kernel_direct(spike_seq, W)
    return _kernel_pe(spike_seq, W)


def _kernel_pe(spike_seq: np.ndarray, W: np.ndarray) -> np.ndarray:
    nc = _get_program("pe")

    # lhsT rows: w1 terms first, then w2 terms — this accumulation order was
    # validated to reproduce the reference's f32 `s0*w1 + s1*w2` exactly.
    w1h, w1m, w1l = _split3_bf16(W[:, 0])
    w2h, w2m, w2l = _split3_bf16(W[:, 1])
    w6 = np.stack([w1h, w1m, w1l, w2h, w2m, w2l]).astype(ml_dtypes.bfloat16)

    in_maps = []
    for c in range(NCORES):
        sl = spike_seq[:, c * BP : (c + 1) * BP, :]          # [T, BP, 2]
        s0 = sl[:, :, 0].reshape(T * BP)
        s1 = sl[:, :, 1].reshape(T * BP)
        rhs6 = np.stack([s0, s0, s0, s1, s1, s1]).astype(ml_dtypes.bfloat16)
        in_maps.append({"rhs6": rhs6, "w6": w6})

    res = run_bass_kernel_spmd(nc, in_maps, core_ids=list(range(NCORES)))

    out = np.empty((T, B, N), dtype=np.float32)
    for c in range(NCORES):
        oc = res.results[c]["out"]                           # [N, T, BP]
        out[:, c * BP : (c + 1) * BP, :] = oc.transpose(1, 2, 0)
    return out


def _kernel_direct(spike_seq: np.ndarray, W: np.ndarray) -> np.ndarray:
    nc = _get_program("direct")
    w1c = np.float32(W[0, 0])
    w2 = W[:, 1]
    # w2b[p, f] = w2[(p//BP... p//64)*64 + f]; rows identical within a half
    w2b = np.concatenate(
        [np.tile(w2[:64], (64, 1)), np.tile(w2[64:], (64, 1))], axis=0
    ).astype(np.float32)

    in_maps = []
    for c in range(NCORES):
        sl = spike_seq[:, c * BP : (c + 1) * BP, :]          # [T, BP, 2]
        s1t = np.tile(sl[:, :, 1].T, (2, 1))                 # [128, T]
        s0t = np.tile((sl[:, :, 0] * w1c).T, (2, 1))         # [128, T] exact
        scols = np.concatenate([s1t, s0t], axis=1).astype(np.float32)
        in_maps.append({"scols": scols, "w2b": w2b})

    res = run_bass_kernel_spmd(nc, in_maps, core_ids=list(range(NCORES)))

    out = np.empty((T, B, N), dtype=np.float32)
    for c in range(NCORES):
        oc = res.results[c]["out"]                           # [128=(h,b), T, BP]
        # full[t, c*BP + b, h*64 + f] = oc[h*64+b, t, f]
        out[:, c * BP : (c + 1) * BP, :] = (
            oc.reshape(2, 64, T, 64).transpose(2, 1, 0, 3).reshape(T, BP, N)
        )
    return out
